# revision 4
# baseline (speedup 1.0000x reference)
"""Trainium2 Bass kernel for the 3-layer MLP encode/decode forward pass.

Computation (B = 65536):
    d_i = pinv(W_i)                       (host, negligible)
    h = lrelu(x @ W1.T)                   [B, 128]
    h = lrelu(h @ W2.T)                   [B, 64]
    h = h @ W3.T                          [B, 16]
    h = lrelu(h @ d3.T)                   [B, 64]   (folded: lrelu((d3@W3) @ h2))
    h = lrelu(h @ d2.T)                   [B, 128]
    out = h @ d1.T                        [B, 784]

Sharding: pure data-parallel — 8 cores x 8192 batch rows each; the tiny
weights (and host-side pinv) are replicated.

Per-core kernel layout: activations are kept feature-major ([feat, batch]) so
the TensorE contracts over features.  x is transposed on-chip via PE
transpose-mode (exact fp32).  The final layer swaps operand roles
(stationary = activation tile, moving = d1.T) which makes the output land
batch-major in PSUM, so no output transpose is needed.  Matmuls run as
float32r (full PE rate at moving-N >= 256).
"""

import numpy as np

B = 65536
N_CORES = 8
B_LOC = B // N_CORES  # 8192
D0, D1, D2, D3 = 784, 128, 64, 16
KCH = 112          # 784 = 7 * 112 contraction chunks for layer 1
NKC = D0 // KCH    # 7
TILE = 512         # moving free dim per matmul (one fp32 PSUM bank)
SUB = 128          # batch sub-tile (partition dim of x / out tiles)
NSUB = TILE // SUB  # 4


def _build_nc(b_loc=B_LOC, mm_dt_name="float32r", last_dt_name="float32r",
              act_name="Lrelu"):
    import concourse.tile as tile
    from concourse import bacc, mybir

    mm_dt = getattr(mybir.dt, mm_dt_name)
    last_dt = getattr(mybir.dt, last_dt_name)
    f32 = mybir.dt.float32
    LRELU = getattr(mybir.ActivationFunctionType, act_name)
    COPY = mybir.ActivationFunctionType.Copy

    nc = bacc.Bacc(trn_type="TRN2", target_bir_lowering=False, debug=False,
                   num_devices=N_CORES)

    x = nc.declare_dram_parameter("x", [b_loc, D0], f32, isOutput=False).ap()
    w1t = nc.declare_dram_parameter("w1t", [D0, D1], mm_dt, isOutput=False).ap()
    w2t = nc.declare_dram_parameter("w2t", [D1, D2], mm_dt, isOutput=False).ap()
    m3t = nc.declare_dram_parameter("m3t", [D2, D2], mm_dt, isOutput=False).ap()
    d2t = nc.declare_dram_parameter("d2t", [D2, D1], mm_dt, isOutput=False).ap()
    d1t = nc.declare_dram_parameter("d1t", [D1, D0], last_dt, isOutput=False).ap()
    ident = nc.declare_dram_parameter("ident", [SUB, SUB], f32, isOutput=False).ap()
    out = nc.declare_dram_parameter("out", [b_loc, D0], f32, isOutput=True).ap()

    n_tiles = b_loc // TILE

    with tile.TileContext(nc, num_cores=N_CORES) as tc:
        with (
            tc.tile_pool(name="consts", bufs=1) as consts,
            tc.tile_pool(name="xin", bufs=8) as xin,
            tc.tile_pool(name="xtp", bufs=14) as xtp,
            tc.tile_pool(name="acts", bufs=2) as acts,
            tc.tile_pool(name="outp", bufs=8) as outp,
            tc.tile_pool(name="psT", bufs=2, space="PSUM") as psT,
            tc.tile_pool(name="psMM", bufs=2, space="PSUM") as psMM,
            tc.tile_pool(name="psO", bufs=2, space="PSUM") as psO,
        ):
            # --- constants ---
            w1t_sb = consts.tile([KCH, NKC, D1], mm_dt)
            nc.sync.dma_start(out=w1t_sb, in_=w1t.rearrange("(c p) m -> p c m", p=KCH))
            w2t_sb = consts.tile([D1, D2], mm_dt)
            nc.sync.dma_start(out=w2t_sb, in_=w2t)
            m3t_sb = consts.tile([D2, D2], mm_dt)
            nc.sync.dma_start(out=m3t_sb, in_=m3t)
            d2t_sb = consts.tile([D2, D1], mm_dt)
            nc.sync.dma_start(out=d2t_sb, in_=d2t)
            d1t_sb = consts.tile([D1, D0], last_dt)
            nc.sync.dma_start(out=d1t_sb, in_=d1t)
            id_sb = consts.tile([SUB, SUB], f32)
            nc.sync.dma_start(out=id_sb, in_=ident)

            for t in range(n_tiles):
                b0 = t * TILE
                # --- load x batch-major ---
                x_sb = []
                for s in range(NSUB):
                    xs = xin.tile([SUB, D0], f32, tag="x")
                    nc.sync.dma_start(
                        out=xs, in_=x[b0 + s * SUB: b0 + (s + 1) * SUB, :])
                    x_sb.append(xs)

                # --- transpose to feature-major: 7 chunks of [112, 512] ---
                xt_sb = []
                for c in range(NKC):
                    tp = psT.tile([KCH, TILE], f32, tag="psT")
                    for s in range(NSUB):
                        nc.tensor.transpose(
                            out=tp[:, s * SUB:(s + 1) * SUB],
                            in_=x_sb[s][:, c * KCH:(c + 1) * KCH],
                            identity=id_sb,
                        )
                    xt = xtp.tile([KCH, TILE], mm_dt, tag="xt")
                    nc.vector.tensor_copy(xt, tp)
                    xt_sb.append(xt)

                # --- L1: h1 = lrelu(W1 @ xT)  [128, 512] ---
                h1_ps = psMM.tile([D1, TILE], f32, tag="mm")
                for c in range(NKC):
                    nc.tensor.matmul(
                        h1_ps,
                        lhsT=w1t_sb[:, c, :],
                        rhs=xt_sb[c],
                        start=(c == 0), stop=(c == NKC - 1))
                h1_sb = acts.tile([D1, TILE], mm_dt, tag="h1")
                nc.scalar.activation(out=h1_sb, in_=h1_ps, func=LRELU, alpha=0.01)

                # --- L2: h2 = lrelu(W2 @ h1)  [64, 512] ---
                h2_ps = psMM.tile([D2, TILE], f32, tag="mm")
                nc.tensor.matmul(h2_ps, lhsT=w2t_sb,
                                 rhs=h1_sb, start=True, stop=True)
                h2_sb = acts.tile([D2, TILE], mm_dt, tag="h2")
                nc.scalar.activation(out=h2_sb, in_=h2_ps, func=LRELU, alpha=0.01)

                # --- L3 folded: g3 = lrelu((d3 @ W3) @ h2)  [64, 512] ---
                g3_ps = psMM.tile([D2, TILE], f32, tag="mm")
                nc.tensor.matmul(g3_ps, lhsT=m3t_sb,
                                 rhs=h2_sb, start=True, stop=True)
                g3_sb = acts.tile([D2, TILE], mm_dt, tag="g3")
                nc.scalar.activation(out=g3_sb, in_=g3_ps, func=LRELU, alpha=0.01)

                # --- L4: g2 = lrelu(d2 @ g3)  [128, 512] ---
                g2_ps = psMM.tile([D1, TILE], f32, tag="mm")
                nc.tensor.matmul(g2_ps, lhsT=d2t_sb,
                                 rhs=g3_sb, start=True, stop=True)
                g2_sb = acts.tile([D1, TILE], last_dt, tag="g2")
                nc.scalar.activation(out=g2_sb, in_=g2_ps, func=LRELU, alpha=0.01)

                # --- L5: out = g2.T @ d1.T, batch-major via stationary swap ---
                HALF = D0 // 2  # 392
                for s in range(NSUB):
                    g2c = g2_sb[:, s * SUB:(s + 1) * SUB]
                    oa = psO.tile([SUB, HALF], f32, tag="oa")
                    ob = psO.tile([SUB, HALF], f32, tag="ob")
                    nc.tensor.matmul(oa, lhsT=g2c,
                                     rhs=d1t_sb[:, :HALF],
                                     start=True, stop=True)
                    nc.tensor.matmul(ob, lhsT=g2c,
                                     rhs=d1t_sb[:, HALF:],
                                     start=True, stop=True)
                    o_sb = outp.tile([SUB, D0], f32, tag="o")
                    nc.scalar.activation(out=o_sb[:, :HALF], in_=oa, func=COPY)
                    nc.scalar.activation(out=o_sb[:, HALF:], in_=ob, func=COPY)
                    nc.sync.dma_start(
                        out=out[b0 + s * SUB: b0 + (s + 1) * SUB, :], in_=o_sb)

    nc.finalize()
    return nc


def _host_weights(W1, W2, W3):
    def pinv(W):
        u, s, vh = np.linalg.svd(W.astype(np.float64), full_matrices=False)
        return (vh.T * (1.0 / s)) @ u.T

    d1, d2, d3 = pinv(W1), pinv(W2), pinv(W3)
    f = np.float32
    return {
        "w1t": np.ascontiguousarray(W1.T, dtype=f),
        "w2t": np.ascontiguousarray(W2.T, dtype=f),
        "m3t": np.ascontiguousarray((d3 @ W3.astype(np.float64)).T, dtype=f),
        "d2t": np.ascontiguousarray(d2.T, dtype=f),
        "d1t": np.ascontiguousarray(d1.T, dtype=f),
        "ident": np.eye(SUB, dtype=f),
    }


_NC_CACHE = {}


def _get_nc(key=("float32r", "float32r")):
    if key not in _NC_CACHE:
        _NC_CACHE[key] = _build_nc(B_LOC, key[0], key[1])
    return _NC_CACHE[key]


def kernel(x, W1, W2, W3):
    from concourse.bass_utils import run_bass_kernel_spmd

    x = np.ascontiguousarray(x, dtype=np.float32)
    w = _host_weights(np.asarray(W1), np.asarray(W2), np.asarray(W3))
    nc = _get_nc()
    in_maps = [
        {"x": x[i * B_LOC:(i + 1) * B_LOC], **w} for i in range(N_CORES)
    ]
    res = run_bass_kernel_spmd(nc, in_maps, core_ids=list(range(N_CORES)))
    return np.concatenate([res.results[i]["out"] for i in range(N_CORES)], axis=0)


# revision 5
# speedup vs baseline: 150.7178x; 150.7178x over previous
"""Trainium2 Bass kernel for the 3-layer MLP encode/decode forward pass.

Computation (B = 65536):
    d_i = pinv(W_i)                       (host, negligible)
    h = lrelu(x @ W1.T)                   [B, 128]
    h = lrelu(h @ W2.T)                   [B, 64]
    h = h @ W3.T                          [B, 16]
    h = lrelu(h @ d3.T)                   [B, 64]   (folded: lrelu((d3@W3) @ h2))
    h = lrelu(h @ d2.T)                   [B, 128]
    out = h @ d1.T                        [B, 784]

Sharding: pure data-parallel — 8 cores x 8192 batch rows each; the tiny
weights (and host-side pinv) are replicated.

Per-core kernel layout: activations are kept feature-major ([feat, batch]) so
the TensorE contracts over features.  x is transposed on-chip via PE
transpose-mode (exact fp32).  The final layer swaps operand roles
(stationary = activation tile, moving = d1.T) which makes the output land
batch-major in PSUM, so no output transpose is needed.  Matmuls run as
float32r (full PE rate at moving-N >= 256).
"""

import numpy as np

B = 65536
N_CORES = 8
B_LOC = B // N_CORES  # 8192
D0, D1, D2, D3 = 784, 128, 64, 16
KCH = 112          # 784 = 7 * 112 contraction chunks for layer 1
NKC = D0 // KCH    # 7
TILE = 512         # moving free dim per matmul (one fp32 PSUM bank)
SUB = 128          # batch sub-tile (partition dim of x / out tiles)
NSUB = TILE // SUB  # 4


def _build_nc(b_loc=B_LOC, mm_dt_name="float32r", last_dt_name="float32r",
              act_name="Lrelu", repeat=1):
    import concourse.tile as tile
    from concourse import bacc, mybir

    mm_dt = getattr(mybir.dt, mm_dt_name)
    last_dt = getattr(mybir.dt, last_dt_name)
    f32 = mybir.dt.float32
    LRELU = getattr(mybir.ActivationFunctionType, act_name)
    COPY = mybir.ActivationFunctionType.Copy

    nc = bacc.Bacc(trn_type="TRN2", target_bir_lowering=False, debug=False,
                   num_devices=N_CORES)

    x = nc.declare_dram_parameter("x", [b_loc, D0], f32, isOutput=False).ap()
    w1t = nc.declare_dram_parameter("w1t", [D0, D1], mm_dt, isOutput=False).ap()
    w2t = nc.declare_dram_parameter("w2t", [D1, D2], mm_dt, isOutput=False).ap()
    m3t = nc.declare_dram_parameter("m3t", [D2, D2], mm_dt, isOutput=False).ap()
    d2t = nc.declare_dram_parameter("d2t", [D2, D1], mm_dt, isOutput=False).ap()
    d1t = nc.declare_dram_parameter("d1t", [D1, D0], last_dt, isOutput=False).ap()
    ident = nc.declare_dram_parameter("ident", [SUB, SUB], f32, isOutput=False).ap()
    out = nc.declare_dram_parameter("out", [b_loc, D0], f32, isOutput=True).ap()

    n_tiles = b_loc // TILE

    with tile.TileContext(nc, num_cores=N_CORES) as tc:
        with (
            tc.tile_pool(name="consts", bufs=1) as consts,
            tc.tile_pool(name="xin", bufs=8) as xin,
            tc.tile_pool(name="xtp", bufs=14) as xtp,
            tc.tile_pool(name="acts", bufs=2) as acts,
            tc.tile_pool(name="outp", bufs=8) as outp,
            tc.tile_pool(name="psT", bufs=2, space="PSUM") as psT,
            tc.tile_pool(name="psMM", bufs=2, space="PSUM") as psMM,
            tc.tile_pool(name="psO", bufs=2, space="PSUM") as psO,
        ):
            # --- constants ---
            w1t_sb = consts.tile([KCH, NKC, D1], mm_dt)
            nc.sync.dma_start(out=w1t_sb, in_=w1t.rearrange("(c p) m -> p c m", p=KCH))
            w2t_sb = consts.tile([D1, D2], mm_dt)
            nc.sync.dma_start(out=w2t_sb, in_=w2t)
            m3t_sb = consts.tile([D2, D2], mm_dt)
            nc.sync.dma_start(out=m3t_sb, in_=m3t)
            d2t_sb = consts.tile([D2, D1], mm_dt)
            nc.sync.dma_start(out=d2t_sb, in_=d2t)
            d1t_sb = consts.tile([D1, D0], last_dt)
            nc.sync.dma_start(out=d1t_sb, in_=d1t)
            id_sb = consts.tile([SUB, SUB], f32)
            nc.sync.dma_start(out=id_sb, in_=ident)

            import contextlib
            rep_ctx = (tc.For_i(0, repeat, 1) if repeat > 1
                       else contextlib.nullcontext())
            with rep_ctx:
              for t in range(n_tiles):
                b0 = t * TILE
                # --- load x batch-major ---
                x_sb = []
                for s in range(NSUB):
                    xs = xin.tile([SUB, D0], f32, tag="x")
                    nc.sync.dma_start(
                        out=xs, in_=x[b0 + s * SUB: b0 + (s + 1) * SUB, :])
                    x_sb.append(xs)

                # --- transpose to feature-major: 7 chunks of [112, 512] ---
                xt_sb = []
                for c in range(NKC):
                    tp = psT.tile([KCH, TILE], f32, tag="psT")
                    for s in range(NSUB):
                        nc.tensor.transpose(
                            out=tp[:, s * SUB:(s + 1) * SUB],
                            in_=x_sb[s][:, c * KCH:(c + 1) * KCH],
                            identity=id_sb,
                        )
                    xt = xtp.tile([KCH, TILE], mm_dt, tag="xt")
                    nc.vector.tensor_copy(xt, tp)
                    xt_sb.append(xt)

                # --- L1: h1 = lrelu(W1 @ xT)  [128, 512] ---
                h1_ps = psMM.tile([D1, TILE], f32, tag="mm")
                for c in range(NKC):
                    nc.tensor.matmul(
                        h1_ps,
                        lhsT=w1t_sb[:, c, :],
                        rhs=xt_sb[c],
                        start=(c == 0), stop=(c == NKC - 1))
                h1_sb = acts.tile([D1, TILE], mm_dt, tag="h1")
                nc.scalar.activation(out=h1_sb, in_=h1_ps, func=LRELU, alpha=0.01)

                # --- L2: h2 = lrelu(W2 @ h1)  [64, 512] ---
                h2_ps = psMM.tile([D2, TILE], f32, tag="mm")
                nc.tensor.matmul(h2_ps, lhsT=w2t_sb,
                                 rhs=h1_sb, start=True, stop=True)
                h2_sb = acts.tile([D2, TILE], mm_dt, tag="h2")
                nc.scalar.activation(out=h2_sb, in_=h2_ps, func=LRELU, alpha=0.01)

                # --- L3 folded: g3 = lrelu((d3 @ W3) @ h2)  [64, 512] ---
                g3_ps = psMM.tile([D2, TILE], f32, tag="mm")
                nc.tensor.matmul(g3_ps, lhsT=m3t_sb,
                                 rhs=h2_sb, start=True, stop=True)
                g3_sb = acts.tile([D2, TILE], mm_dt, tag="g3")
                nc.scalar.activation(out=g3_sb, in_=g3_ps, func=LRELU, alpha=0.01)

                # --- L4: g2 = lrelu(d2 @ g3)  [128, 512] ---
                g2_ps = psMM.tile([D1, TILE], f32, tag="mm")
                nc.tensor.matmul(g2_ps, lhsT=d2t_sb,
                                 rhs=g3_sb, start=True, stop=True)
                g2_sb = acts.tile([D1, TILE], last_dt, tag="g2")
                nc.scalar.activation(out=g2_sb, in_=g2_ps, func=LRELU, alpha=0.01)

                # --- L5: out = g2.T @ d1.T, batch-major via stationary swap ---
                HALF = D0 // 2  # 392
                for s in range(NSUB):
                    g2c = g2_sb[:, s * SUB:(s + 1) * SUB]
                    oa = psO.tile([SUB, HALF], f32, tag="oa")
                    ob = psO.tile([SUB, HALF], f32, tag="ob")
                    nc.tensor.matmul(oa, lhsT=g2c,
                                     rhs=d1t_sb[:, :HALF],
                                     start=True, stop=True)
                    nc.tensor.matmul(ob, lhsT=g2c,
                                     rhs=d1t_sb[:, HALF:],
                                     start=True, stop=True)
                    o_sb = outp.tile([SUB, D0], f32, tag="o")
                    nc.scalar.activation(out=o_sb[:, :HALF], in_=oa, func=COPY)
                    nc.scalar.activation(out=o_sb[:, HALF:], in_=ob, func=COPY)
                    nc.sync.dma_start(
                        out=out[b0 + s * SUB: b0 + (s + 1) * SUB, :], in_=o_sb)

    nc.finalize()
    return nc


def _host_weights(W1, W2, W3):
    def pinv(W):
        u, s, vh = np.linalg.svd(W.astype(np.float64), full_matrices=False)
        return (vh.T * (1.0 / s)) @ u.T

    d1, d2, d3 = pinv(W1), pinv(W2), pinv(W3)
    f = np.float32
    return {
        "w1t": np.ascontiguousarray(W1.T, dtype=f),
        "w2t": np.ascontiguousarray(W2.T, dtype=f),
        "m3t": np.ascontiguousarray((d3 @ W3.astype(np.float64)).T, dtype=f),
        "d2t": np.ascontiguousarray(d2.T, dtype=f),
        "d1t": np.ascontiguousarray(d1.T, dtype=f),
        "ident": np.eye(SUB, dtype=f),
    }


_NC_CACHE = {}


def _get_nc(key=("float32r", "float32r")):
    if key not in _NC_CACHE:
        _NC_CACHE[key] = _build_nc(B_LOC, key[0], key[1])
    return _NC_CACHE[key]


def kernel(x, W1, W2, W3):
    from concourse.bass_utils import run_bass_kernel_spmd

    x = np.ascontiguousarray(x, dtype=np.float32)
    w = _host_weights(np.asarray(W1), np.asarray(W2), np.asarray(W3))
    nc = _get_nc()
    in_maps = [
        {"x": x[i * B_LOC:(i + 1) * B_LOC], **w} for i in range(N_CORES)
    ]
    res = run_bass_kernel_spmd(nc, in_maps, core_ids=list(range(N_CORES)))
    return np.concatenate([res.results[i]["out"] for i in range(N_CORES)], axis=0)
